# revision 17
# baseline (speedup 1.0000x reference)
"""GQA attention kernel for 8 Trainium2 NeuronCores.

Sharding: 2-way data parallel over batch x 4-way tensor parallel over heads.
Each core handles one batch element and 8 q-heads (2 kv-heads). The o-proj
partial outputs are summed on the host (replaces the all-reduce).

Per-core layout strategy: everything is kept transposed ([feature, seq]) so
every matmul consumes operands directly with the contraction dim on SBUF
partitions and no on-device transposes of activations are needed:
  Q^T = Wq_s^T @ x^T         (lhsT = Wq_s tiles, rhs = x^T tiles)
  S^T[k,q] = K^T_tile^T @ Q^T (k on partitions -> softmax denom via matmul)
  Y^T[d,q] = V_aug^T @ exp(S^T)  (V augmented with a ones column gives the
                                  softmax denominator for free in row 64)
  O^T = Wo_s^T @ (Y^T / Z)

Pipeline structure (v4):
 - single in-order PE stream: proj chunk 0,1 / attn(qc0) interleaved with
   proj chunks 2,3 / attn(qc1..3) with o-proj(qc-1) m-blocks sprinkled
   between attention iterations, so the PE never drains while the scalar
   engine works through the exp stream (exp is the attention-phase pacer).
 - AV matmuls run one kt behind the scores matmuls: the in-order PE queue
   never waits on the exp of the tile it just produced.
 - causality at 128-key granularity: diagonal blocks only compute the
   q >= key part (column-sliced matmul/exp/mask), upper-left full blocks.
 - softmax normalization fully off the PE queue: approx reciprocal on DVE,
   partition broadcast on gpsimd, fused multiply from PSUM on DVE.
 - PSUM: shared 2-buf accumulator pool (proj chains, rope, V-transpose,
   o-proj) + 2x2-bank score tiles + 2 AV accumulators = 8 banks.
"""

import numpy as np

B, T, C, D = 2, 2048, 2048, 64
KT = 16          # contraction tiles over C
NCH = 4          # 512-wide chunks over T
NQ = 512
ROPE_BASE = 10000.0
LPERM = [0, 4, 1, 5, 2, 6, 3, 7]  # local head order: pair j = (j, j+4)

_CACHE = {}


def _build_nc():
    import concourse.bass as bass  # noqa: F401
    import concourse.mybir as mybir
    from concourse import bacc
    from concourse.tile import TileContext
    from concourse.masks import make_identity

    F32 = mybir.dt.float32
    F16 = mybir.dt.float16
    AF = mybir.ActivationFunctionType

    nc = bacc.Bacc(None, target_bir_lowering=False, debug=True)
    # all weight/activation inputs are pre-arranged on the host into the
    # exact SBUF layout so every DMA is a fully contiguous big-run copy
    xT = nc.dram_tensor("xT", [128, NCH, KT, NQ], F16, kind="ExternalInput")
    wq = nc.dram_tensor("wq", [128, KT * 512], F16, kind="ExternalInput")
    wk = nc.dram_tensor("wk", [128, KT * 128], F16, kind="ExternalInput")
    wv = nc.dram_tensor("wv", [128, KT * 128], F16, kind="ExternalInput")
    wo = nc.dram_tensor("wo", [128, 4 * 2048], F16, kind="ExternalInput")
    cosf = nc.dram_tensor("cosf", [128, T], F16, kind="ExternalInput")
    sinf = nc.dram_tensor("sinf", [128, T], F16, kind="ExternalInput")
    perm = nc.dram_tensor("perm", [128, 128], F16, kind="ExternalInput")
    outT = nc.dram_tensor("outT", [C, T], F16, kind="ExternalOutput")

    with TileContext(nc) as tc:
        with (
            tc.tile_pool(name="const", bufs=1) as cpool,
            tc.tile_pool(name="big", bufs=1) as bpool,
        ):
            wq_sb = cpool.tile([128, KT * 512], F16, tag="wq")
            wk_sb = cpool.tile([128, KT * 128], F16, tag="wk")
            wv_sb = cpool.tile([128, KT * 128], F16, tag="wv")
            cos_sb = cpool.tile([128, T], F16, tag="cos")
            sin_sb = cpool.tile([128, T], F16, tag="sin")
            perm_sb = cpool.tile([128, 128], F16, tag="perm")
            ident = cpool.tile([128, 128], F16, tag="ident")
            wo_sb = cpool.tile([128, 4 * 2048], F16, tag="wo")

            nc.sync.dma_start(out=wk_sb[:], in_=wk[:, :])
            nc.sync.dma_start(out=wv_sb[:], in_=wv[:, :])
            nc.sync.dma_start(out=wq_sb[:], in_=wq[:, :])
            nc.sync.dma_start(out=perm_sb[:], in_=perm[:, :])
            make_identity(nc, ident[:])
            ebias = cpool.tile([128, 1], F32, tag="ebias")
            nc.vector.memset(ebias[:], -8.0)

            # persistent transposed activations
            qt = [bpool.tile([128, T], F16, tag=f"qt{j}", name=f"qt{j}") for j in range(4)]
            kt_sb = bpool.tile([128, T], F16, tag="ktT")
            vnat = bpool.tile([128, KT * 130], F16, tag="vnat")
            nc.vector.memset(vnat[:], 1.0)

            with (
                tc.tile_pool(name="xs", bufs=4) as xpool,
                tc.tile_pool(name="acc", bufs=2, space="PSUM") as acc,
                tc.tile_pool(name="sps", bufs=2, space="PSUM") as spool,
                tc.tile_pool(name="aps", bufs=1, space="PSUM") as apool,
                tc.tile_pool(name="rtmp", bufs=4) as rtpool,
                tc.tile_pool(name="vtt", bufs=2) as vtpool,
                tc.tile_pool(name="esb", bufs=4) as epool,
                tc.tile_pool(name="ysb", bufs=2) as ypool,
                tc.tile_pool(name="zsb", bufs=2) as zpool,
                tc.tile_pool(name="stg", bufs=3) as stpool,
            ):
                yq_all = [None] * NCH

                def proj_chunk(n):
                    nsl = slice(n * NQ, (n + 1) * NQ)
                    xsb = []
                    for half in range(2):
                        xh = xpool.tile([128, 8 * NQ], F16, tag="xsb")
                        nc.sync.dma_start(
                            out=xh[:].rearrange("p (kt t) -> p kt t", kt=8),
                            in_=xT[:, n, half * 8:(half + 1) * 8, :],
                        )
                        xsb.append(xh)
                    if n == 0:
                        # needed only from the rope phase / attention onward;
                        # issued after the hot-path DMAs so they don't
                        # contend with wk/wv/wq/x during startup
                        nc.sync.dma_start(out=cos_sb[:], in_=cosf[:, :])
                        nc.sync.dma_start(out=sin_sb[:], in_=sinf[:, :])
                        nc.sync.dma_start(out=wo_sb[:], in_=wo[:, :])
                    vt_sb = vtpool.tile([128, NQ], F16, tag="vtT")
                    # K and V chains first: their weights arrive first and V
                    # feeds the vnat transposes
                    for m in (4, 5, 0, 1, 2, 3):
                        ps = acc.tile([128, NQ], F32, tag="ps")
                        for kt in range(KT):
                            if m < 4:
                                w_ap = wq_sb[:, kt * 512 + m * 128: kt * 512 + (m + 1) * 128]
                            elif m == 4:
                                w_ap = wk_sb[:, kt * 128:(kt + 1) * 128]
                            else:
                                w_ap = wv_sb[:, kt * 128:(kt + 1) * 128]
                            x_ap = xsb[kt // 8][:, (kt % 8) * NQ:(kt % 8 + 1) * NQ]
                            nc.tensor.matmul(
                                ps[:], w_ap, x_ap,
                                start=(kt == 0), stop=(kt == KT - 1),
                                skip_group_check=True,
                            )
                        if m < 4:
                            nc.scalar.copy(qt[m][:, nsl], ps[:])
                        elif m == 4:
                            nc.scalar.copy(kt_sb[:, nsl], ps[:])
                        else:
                            nc.scalar.copy(vt_sb[:], ps[:])
                        if m == 5:
                            # V chunk -> vnat transposed blocks
                            for j in range(4):
                                g = 4 * n + j
                                tp = acc.tile([128, NQ], F32, tag="ps")
                                tpv = tp[:, 0:64].bitcast(F16)
                                nc.tensor.transpose(
                                    tpv,
                                    vt_sb[:, j * 128:(j + 1) * 128], ident[:])
                                nc.vector.tensor_copy(
                                    vnat[:, g * 130: g * 130 + 64], tpv[:, 0:64])
                                nc.vector.tensor_copy(
                                    vnat[:, g * 130 + 65: g * 130 + 129],
                                    tpv[:, 64:128])
                    # rope on this chunk of q0..q3 and k
                    for tile in [qt[0], qt[1], qt[2], qt[3], kt_sb]:
                        qs = acc.tile([128, NQ], F32, tag="ps")
                        nc.tensor.matmul(qs[:], perm_sb[:], tile[:, nsl],
                                         start=True, stop=True,
                                         skip_group_check=True)
                        t1 = rtpool.tile([128, NQ], F16, tag="t1")
                        t2 = rtpool.tile([128, NQ], F16, tag="t2")
                        nc.vector.tensor_mul(t1[:], tile[:, nsl], cos_sb[:, nsl])
                        nc.vector.tensor_mul(t2[:], qs[:], sin_sb[:, nsl])
                        nc.vector.tensor_add(tile[:, nsl], t1[:], t2[:])

                def oproj_block(qc, m):
                    qsl = slice(qc * NQ, (qc + 1) * NQ)
                    ops = acc.tile([128, NQ], F32, tag="ps")
                    for g in range(4):
                        nc.tensor.matmul(
                            ops[:],
                            wo_sb[:, g * 2048 + m * 128: g * 2048 + (m + 1) * 128],
                            yq_all[qc][g][:], start=(g == 0), stop=(g == 3),
                            skip_group_check=True)
                    st = stpool.tile([128, NQ], F16, tag="st")
                    if qc == NCH - 1:
                        # final chunk: exp stream is done, the scalar engine
                        # is idle -- keep the tail off the vector engine
                        nc.scalar.copy(st[:], ops[:])
                    else:
                        nc.vector.tensor_copy(st[:], ops[:])
                    nc.sync.dma_start(
                        out=outT[m * 128:(m + 1) * 128, qsl], in_=st[:])

                def attn_pj(qc, pj, on_iter=None):
                    qbase = qc * NQ
                    kt_hi = 4 * (qc + 1)
                    yaugA = apool.tile([65, NQ], F32, tag="yaugA",
                                       name=f"yaugA_{qc}_{pj}")
                    yaugB = apool.tile([65, NQ], F32, tag="yaugB",
                                       name=f"yaugB_{qc}_{pj}")

                    def av_mms(eab, kt, off):
                        nc.tensor.matmul(
                            yaugA[:, off:NQ], vnat[:, kt * 130: kt * 130 + 65],
                            eab[:, off:NQ],
                            start=(kt == 0), stop=(kt == kt_hi - 1),
                            skip_group_check=True)
                        nc.tensor.matmul(
                            yaugB[:, off:NQ],
                            vnat[:, kt * 130 + 65: kt * 130 + 130],
                            eab[:, NQ + off:2 * NQ],
                            start=(kt == 0), stop=(kt == kt_hi - 1),
                            skip_group_check=True)

                    # AV runs one kt behind the scores so the in-order PE
                    # queue never waits on the exp of the tile just produced
                    prev = None
                    for kt in range(kt_hi):
                        diag = kt * 128 >= qbase
                        off = max(0, kt * 128 - qbase)
                        w = NQ - off
                        sab = spool.tile([128, 2 * NQ], F32, tag="sab",
                                         name=f"sab_{qc}_{pj}_{kt}")
                        nc.tensor.matmul(
                            sab[:, off:NQ],
                            kt_sb[0:64, kt * 128:(kt + 1) * 128],
                            qt[pj][0:64, qbase + off:qbase + NQ],
                            start=True, stop=True, skip_group_check=True)
                        nc.tensor.matmul(
                            sab[:, NQ + off:2 * NQ],
                            kt_sb[64:128, kt * 128:(kt + 1) * 128],
                            qt[pj][64:128, qbase + off:qbase + NQ],
                            start=True, stop=True, skip_group_check=True)
                        if prev is not None:
                            av_mms(*prev)
                        eab = epool.tile([128, 2 * NQ], F16, tag="eab",
                                         name=f"eab_{qc}_{pj}_{kt}")
                        eview = eab[:].rearrange("p (h q) -> p h q", h=2)[:, :, off:NQ]
                        sview = sab[:].rearrange("p (h q) -> p h q", h=2)[:, :, off:NQ]
                        nc.scalar.activation(eview, sview, AF.Exp,
                                             scale=0.125, bias=ebias[:])
                        if diag:
                            nc.gpsimd.affine_select(
                                out=eview, in_=eview,
                                compare_op=mybir.AluOpType.is_ge,
                                fill=0.0,
                                base=0,
                                channel_multiplier=-1,
                                pattern=[[0, 2], [1, w]],
                            )
                        prev = (eab, kt, off)
                        if on_iter is not None:
                            on_iter()
                    av_mms(*prev)
                    # normalize pj: 1/Z on DVE, broadcast on gpsimd
                    yq = yq_all[qc]
                    zinvA = zpool.tile([1, NQ], F32, tag="zinvA",
                                       name=f"zinvA_{qc}_{pj}")
                    zinvB = zpool.tile([1, NQ], F32, tag="zinvB",
                                       name=f"zinvB_{qc}_{pj}")
                    zinvhA = zpool.tile([1, NQ], F16, tag="zinvhA",
                                        name=f"zinvhA_{qc}_{pj}")
                    zinvhB = zpool.tile([1, NQ], F16, tag="zinvhB",
                                        name=f"zinvhB_{qc}_{pj}")
                    znA = zpool.tile([1, NQ], F32, tag="znA",
                                     name=f"znA_{qc}_{pj}")
                    znB = zpool.tile([1, NQ], F32, tag="znB",
                                     name=f"znB_{qc}_{pj}")
                    nc.vector.tensor_copy(znA[:], yaugA[64:65, :])
                    nc.vector.tensor_copy(znB[:], yaugB[64:65, :])
                    nc.vector.reciprocal_approx_fast(zinvA[:], znA[:])
                    nc.vector.reciprocal_approx_fast(zinvB[:], znB[:])
                    nc.vector.tensor_copy(zinvhA[:], zinvA[:])
                    nc.vector.tensor_copy(zinvhB[:], zinvB[:])
                    zbA = zpool.tile([64, NQ], F16, tag="zbA",
                                     name=f"zbA_{qc}_{pj}")
                    zbB = zpool.tile([64, NQ], F16, tag="zbB",
                                     name=f"zbB_{qc}_{pj}")
                    nc.gpsimd.partition_broadcast(zbA[:], zinvhA[:],
                                                  channels=64)
                    nc.gpsimd.partition_broadcast(zbB[:], zinvhB[:],
                                                  channels=64)
                    nc.vector.tensor_mul(yq[pj][0:64, :], yaugA[0:64, :],
                                         zbA[:])
                    nc.vector.tensor_mul(yq[pj][64:128, :], yaugB[0:64, :],
                                         zbB[:])

                def new_yq(qc):
                    yq_all[qc] = [ypool.tile([128, NQ], F16, tag=f"yq{g}",
                                             name=f"yq{g}_{qc}")
                                  for g in range(4)]

                # ---- emission schedule ----
                proj_chunk(0)
                proj_chunk(1)
                new_yq(0)
                attn_pj(0, 0)
                proj_chunk(2)
                attn_pj(0, 1)
                proj_chunk(3)
                attn_pj(0, 2)
                attn_pj(0, 3)
                for qc in range(1, NCH):
                    new_yq(qc)
                    state = {"it": 0, "m": 0}
                    sprinkle_every = qc + 1

                    def on_iter(qc=qc, state=state, se=sprinkle_every):
                        state["it"] += 1
                        if state["m"] < 16 and state["it"] % se == 0:
                            oproj_block(qc - 1, state["m"])
                            state["m"] += 1

                    for pj in range(4):
                        attn_pj(qc, pj, on_iter)
                    while state["m"] < 16:
                        oproj_block(qc - 1, state["m"])
                        state["m"] += 1
                for m in range(16):
                    oproj_block(NCH - 1, m)
    nc.finalize()
    return nc


def _rope_tables():
    inv = 1.0 / (ROPE_BASE ** (np.arange(0, D, 2, dtype=np.float32) / D))
    fr_ = np.arange(T, dtype=np.float32)[:, None] * inv[None, :]
    cosT = np.cos(fr_).T.astype(np.float32)
    sinT = np.sin(fr_).T.astype(np.float32)
    cosfull = np.ascontiguousarray(np.tile(cosT, (4, 1)))
    sinfull = np.ascontiguousarray(np.concatenate([-sinT, sinT, -sinT, sinT]))
    return cosfull, sinfull


def _perm_matrix():
    p = np.zeros((128, 128), dtype=np.float32)
    for i in range(128):
        j = i + 32 if (i % 64) < 32 else i - 32
        p[i, j] = 1.0
    return p


def _get_nc():
    if "nc" not in _CACHE:
        _CACHE["nc"] = _build_nc()
    return _CACHE["nc"]


def make_in_maps(x, Wq, Wk, Wv, Wo):
    cosfull, sinfull = _rope_tables()
    permm = _perm_matrix()
    in_maps = []
    def sb_layout(w):
        # [KT*128, M] -> [128, KT*M]: host-side version of the SBUF layout
        kt = w.shape[0] // 128
        return np.ascontiguousarray(
            w.reshape(kt, 128, -1).transpose(1, 0, 2).reshape(128, -1))

    xts = {}
    for b in range(B):
        # [C, T] -> [128, NCH, KT, NQ] so each chunk DMA is contiguous
        xt = x[b].T.reshape(KT, 128, NCH, NQ).transpose(1, 2, 0, 3)
        xts[b] = np.ascontiguousarray(xt).astype(np.float16)
    for c in range(8):
        b, r = divmod(c, 4)
        qcols = np.concatenate(
            [np.arange(64 * (8 * r + h), 64 * (8 * r + h) + 64) for h in LPERM])
        in_maps.append({
            "xT": xts[b],
            "wq": sb_layout(Wq[:, qcols]).astype(np.float16),
            "wk": sb_layout(Wk[:, 128 * r:128 * (r + 1)]).astype(np.float16),
            "wv": sb_layout(Wv[:, 128 * r:128 * (r + 1)]).astype(np.float16),
            "wo": sb_layout(Wo[qcols, :]).astype(np.float16),
            "cosf": cosfull.astype(np.float16),
            "sinf": sinfull.astype(np.float16),
            "perm": permm.astype(np.float16),
        })
    return in_maps


def run(x, Wq, Wk, Wv, Wo, **spmd_kwargs):
    from concourse.bass_utils import run_bass_kernel_spmd

    nc = _get_nc()
    in_maps = make_in_maps(x, Wq, Wk, Wv, Wo)
    res = run_bass_kernel_spmd(nc, in_maps, list(range(8)), **spmd_kwargs)
    out = np.zeros((B, T, C), dtype=np.float32)
    for c in range(8):
        out[c // 4] += res.results[c]["outT"].T.astype(np.float32)
    return out, res


def kernel(**inputs):
    x = np.asarray(inputs["x"], dtype=np.float32)
    Wq = np.asarray(inputs["Wq"], dtype=np.float32)
    Wk = np.asarray(inputs["Wk"], dtype=np.float32)
    Wv = np.asarray(inputs["Wv"], dtype=np.float32)
    Wo = np.asarray(inputs["Wo"], dtype=np.float32)
    out, _ = run(x, Wq, Wk, Wv, Wo)
    return out


# revision 21
# speedup vs baseline: 1.1825x; 1.1825x over previous
"""GQA attention kernel for 8 Trainium2 NeuronCores.

Sharding: 2-way data parallel over batch x 4-way tensor parallel over heads.
Each core handles one batch element and 8 q-heads (2 kv-heads). The o-proj
partial outputs are summed on the host (replaces the all-reduce).

Per-core layout strategy: everything is kept transposed ([feature, seq]) so
every matmul consumes operands directly with the contraction dim on SBUF
partitions and no on-device transposes of activations are needed:
  Q^T = Wq_s^T @ x^T         (lhsT = Wq_s tiles, rhs = x^T tiles)
  S^T[k,q] = K^T_tile^T @ Q^T (k on partitions -> softmax denom via matmul)
  Y^T[d,q] = V_aug^T @ exp(S^T)  (V augmented with a ones column gives the
                                  softmax denominator for free in row 64)
  O^T = Wo_s^T @ (Y^T / Z)

Pipeline structure (v4):
 - single in-order PE stream: proj chunk 0,1 / attn(qc0) interleaved with
   proj chunks 2,3 / attn(qc1..3) with o-proj(qc-1) m-blocks sprinkled
   between attention iterations, so the PE never drains while the scalar
   engine works through the exp stream (exp is the attention-phase pacer).
 - AV matmuls run one kt behind the scores matmuls: the in-order PE queue
   never waits on the exp of the tile it just produced.
 - causality at 128-key granularity: diagonal blocks only compute the
   q >= key part (column-sliced matmul/exp/mask), upper-left full blocks.
 - softmax normalization fully off the PE queue: approx reciprocal on DVE,
   partition broadcast on gpsimd, fused multiply from PSUM on DVE.
 - PSUM: shared 2-buf accumulator pool (proj chains, rope, V-transpose,
   o-proj) + 2x2-bank score tiles + 2 AV accumulators = 8 banks.
"""

import numpy as np

B, T, C, D = 2, 2048, 2048, 64
KT = 16          # contraction tiles over C
NCH = 4          # 512-wide chunks over T
NQ = 512
ROPE_BASE = 10000.0
LPERM = [0, 4, 1, 5, 2, 6, 3, 7]  # local head order: pair j = (j, j+4)

_CACHE = {}


def _build_nc():
    import concourse.bass as bass  # noqa: F401
    import concourse.mybir as mybir
    from concourse import bacc
    from concourse.tile import TileContext
    from concourse.masks import make_identity

    F32 = mybir.dt.float32
    F16 = mybir.dt.float16
    AF = mybir.ActivationFunctionType

    nc = bacc.Bacc(None, target_bir_lowering=False, debug=True)
    # all weight/activation inputs are pre-arranged on the host into the
    # exact SBUF layout so every DMA is a fully contiguous big-run copy
    xT = nc.dram_tensor("xT", [128, NCH, KT, NQ], F16, kind="ExternalInput")
    wq = nc.dram_tensor("wq", [128, KT * 512], F16, kind="ExternalInput")
    wk = nc.dram_tensor("wk", [128, KT * 128], F16, kind="ExternalInput")
    wv = nc.dram_tensor("wv", [128, KT * 128], F16, kind="ExternalInput")
    wo = nc.dram_tensor("wo", [128, 4 * 2048], F16, kind="ExternalInput")
    cosf = nc.dram_tensor("cosf", [128, T], F16, kind="ExternalInput")
    sinf = nc.dram_tensor("sinf", [128, T], F16, kind="ExternalInput")
    perm = nc.dram_tensor("perm", [128, 128], F16, kind="ExternalInput")
    outT = nc.dram_tensor("outT", [C, T], F16, kind="ExternalOutput")

    with TileContext(nc) as tc:
        with (
            tc.tile_pool(name="const", bufs=1) as cpool,
            tc.tile_pool(name="big", bufs=1) as bpool,
        ):
            wq_sb = cpool.tile([128, KT * 512], F16, tag="wq")
            wk_sb = cpool.tile([128, KT * 128], F16, tag="wk")
            wv_sb = cpool.tile([128, KT * 128], F16, tag="wv")
            cos_sb = cpool.tile([128, T], F16, tag="cos")
            sin_sb = cpool.tile([128, T], F16, tag="sin")
            perm_sb = cpool.tile([128, 128], F16, tag="perm")
            ident = cpool.tile([128, 128], F16, tag="ident")
            wo_sb = cpool.tile([128, 4 * 2048], F16, tag="wo")

            nc.sync.dma_start(out=wk_sb[:], in_=wk[:, :])
            nc.sync.dma_start(out=wv_sb[:], in_=wv[:, :])
            make_identity(nc, ident[:])
            ebias = cpool.tile([128, 1], F32, tag="ebias")
            nc.vector.memset(ebias[:], -8.0)

            # persistent transposed activations
            qt = [bpool.tile([128, T], F16, tag=f"qt{j}", name=f"qt{j}") for j in range(4)]
            kt_sb = bpool.tile([128, T], F16, tag="ktT")
            vnat = bpool.tile([128, KT * 130], F16, tag="vnat")
            nc.vector.memset(vnat[:], 1.0)

            with (
                tc.tile_pool(name="xs", bufs=4) as xpool,
                tc.tile_pool(name="acc", bufs=2, space="PSUM") as acc,
                tc.tile_pool(name="sps", bufs=2, space="PSUM") as spool,
                tc.tile_pool(name="aps", bufs=1, space="PSUM") as apool,
                tc.tile_pool(name="rtmp", bufs=4) as rtpool,
                tc.tile_pool(name="vtt", bufs=2) as vtpool,
                tc.tile_pool(name="esb", bufs=4) as epool,
                tc.tile_pool(name="ysb", bufs=2) as ypool,
                tc.tile_pool(name="zsb", bufs=2) as zpool,
                tc.tile_pool(name="stg", bufs=3) as stpool,
            ):
                yq_all = [None] * NCH

                def load_x(n):
                    xsb = []
                    for half in range(2):
                        xh = xpool.tile([128, 8 * NQ], F16, tag="xsb")
                        nc.sync.dma_start(
                            out=xh[:].rearrange("p (kt t) -> p kt t", kt=8),
                            in_=xT[:, n, half * 8:(half + 1) * 8, :],
                        )
                        xsb.append(xh)
                    return xsb

                def proj_chunk(n, xsb):
                    nsl = slice(n * NQ, (n + 1) * NQ)
                    if n == 0:
                        # everything below is needed only from the rope
                        # phase / attention onward; issued after the
                        # hot-path wk/wv/x DMAs so startup isn't delayed
                        nc.sync.dma_start(out=wq_sb[:], in_=wq[:, :])
                        nc.sync.dma_start(out=perm_sb[:], in_=perm[:, :])
                        nc.sync.dma_start(out=cos_sb[:], in_=cosf[:, :])
                        nc.sync.dma_start(out=sin_sb[:], in_=sinf[:, :])
                        nc.sync.dma_start(out=wo_sb[:], in_=wo[:, :])
                    vt_sb = vtpool.tile([128, NQ], F16, tag="vtT")
                    # K and V chains first: their weights arrive first and V
                    # feeds the vnat transposes
                    for m in (4, 5, 0, 1, 2, 3):
                        ps = acc.tile([128, NQ], F32, tag="ps")
                        for kt in range(KT):
                            if m < 4:
                                w_ap = wq_sb[:, kt * 512 + m * 128: kt * 512 + (m + 1) * 128]
                            elif m == 4:
                                w_ap = wk_sb[:, kt * 128:(kt + 1) * 128]
                            else:
                                w_ap = wv_sb[:, kt * 128:(kt + 1) * 128]
                            x_ap = xsb[kt // 8][:, (kt % 8) * NQ:(kt % 8 + 1) * NQ]
                            nc.tensor.matmul(
                                ps[:], w_ap, x_ap,
                                start=(kt == 0), stop=(kt == KT - 1),
                                skip_group_check=True,
                            )
                        if m < 4:
                            nc.scalar.copy(qt[m][:, nsl], ps[:])
                        elif m == 4:
                            nc.scalar.copy(kt_sb[:, nsl], ps[:])
                        else:
                            nc.scalar.copy(vt_sb[:], ps[:])
                        if m == 5:
                            # V chunk -> vnat transposed blocks
                            for j in range(4):
                                g = 4 * n + j
                                tp = acc.tile([128, NQ], F32, tag="ps")
                                tpv = tp[:, 0:64].bitcast(F16)
                                nc.tensor.transpose(
                                    tpv,
                                    vt_sb[:, j * 128:(j + 1) * 128], ident[:])
                                nc.vector.tensor_copy(
                                    vnat[:, g * 130: g * 130 + 64], tpv[:, 0:64])
                                nc.vector.tensor_copy(
                                    vnat[:, g * 130 + 65: g * 130 + 129],
                                    tpv[:, 64:128])
                    # rope on this chunk of q0..q3 and k
                    for tile in [qt[0], qt[1], qt[2], qt[3], kt_sb]:
                        qs = acc.tile([128, NQ], F32, tag="ps")
                        nc.tensor.matmul(qs[:], perm_sb[:], tile[:, nsl],
                                         start=True, stop=True,
                                         skip_group_check=True)
                        t1 = rtpool.tile([128, NQ], F16, tag="t1")
                        t2 = rtpool.tile([128, NQ], F16, tag="t2")
                        nc.vector.tensor_mul(t1[:], tile[:, nsl], cos_sb[:, nsl])
                        nc.vector.tensor_mul(t2[:], qs[:], sin_sb[:, nsl])
                        nc.vector.tensor_add(tile[:, nsl], t1[:], t2[:])

                def oproj_block(qc, m):
                    qsl = slice(qc * NQ, (qc + 1) * NQ)
                    ops = acc.tile([128, NQ], F32, tag="ps")
                    for g in range(4):
                        nc.tensor.matmul(
                            ops[:],
                            wo_sb[:, g * 2048 + m * 128: g * 2048 + (m + 1) * 128],
                            yq_all[qc][g][:], start=(g == 0), stop=(g == 3),
                            skip_group_check=True)
                    st = stpool.tile([128, NQ], F16, tag="st")
                    nc.vector.tensor_copy(st[:], ops[:])
                    nc.sync.dma_start(
                        out=outT[m * 128:(m + 1) * 128, qsl], in_=st[:])

                def attn_pj(qc, pj, on_iter=None):
                    qbase = qc * NQ
                    kt_hi = 4 * (qc + 1)
                    yaugA = apool.tile([65, NQ], F32, tag="yaugA",
                                       name=f"yaugA_{qc}_{pj}")
                    yaugB = apool.tile([65, NQ], F32, tag="yaugB",
                                       name=f"yaugB_{qc}_{pj}")

                    def av_mms(eab, kt, off):
                        nc.tensor.matmul(
                            yaugA[:, off:NQ], vnat[:, kt * 130: kt * 130 + 65],
                            eab[:, off:NQ],
                            start=(kt == 0), stop=(kt == kt_hi - 1),
                            skip_group_check=True)
                        nc.tensor.matmul(
                            yaugB[:, off:NQ],
                            vnat[:, kt * 130 + 65: kt * 130 + 130],
                            eab[:, NQ + off:2 * NQ],
                            start=(kt == 0), stop=(kt == kt_hi - 1),
                            skip_group_check=True)

                    # AV runs one kt behind the scores so the in-order PE
                    # queue never waits on the exp of the tile just produced
                    prev = None
                    for kt in range(kt_hi):
                        diag = kt * 128 >= qbase
                        off = max(0, kt * 128 - qbase)
                        w = NQ - off
                        sab = spool.tile([128, 2 * NQ], F32, tag="sab",
                                         name=f"sab_{qc}_{pj}_{kt}")
                        nc.tensor.matmul(
                            sab[:, off:NQ],
                            kt_sb[0:64, kt * 128:(kt + 1) * 128],
                            qt[pj][0:64, qbase + off:qbase + NQ],
                            start=True, stop=True, skip_group_check=True)
                        nc.tensor.matmul(
                            sab[:, NQ + off:2 * NQ],
                            kt_sb[64:128, kt * 128:(kt + 1) * 128],
                            qt[pj][64:128, qbase + off:qbase + NQ],
                            start=True, stop=True, skip_group_check=True)
                        if prev is not None:
                            av_mms(*prev)
                        eab = epool.tile([128, 2 * NQ], F16, tag="eab",
                                         name=f"eab_{qc}_{pj}_{kt}")
                        eview = eab[:].rearrange("p (h q) -> p h q", h=2)[:, :, off:NQ]
                        sview = sab[:].rearrange("p (h q) -> p h q", h=2)[:, :, off:NQ]
                        nc.scalar.activation(eview, sview, AF.Exp,
                                             scale=0.125, bias=ebias[:])
                        if diag:
                            nc.gpsimd.affine_select(
                                out=eview, in_=eview,
                                compare_op=mybir.AluOpType.is_ge,
                                fill=0.0,
                                base=0,
                                channel_multiplier=-1,
                                pattern=[[0, 2], [1, w]],
                            )
                        prev = (eab, kt, off)
                        if on_iter is not None:
                            on_iter()
                    av_mms(*prev)
                    # normalize pj: 1/Z on DVE, broadcast on gpsimd
                    yq = yq_all[qc]
                    zinvA = zpool.tile([1, NQ], F32, tag="zinvA",
                                       name=f"zinvA_{qc}_{pj}")
                    zinvB = zpool.tile([1, NQ], F32, tag="zinvB",
                                       name=f"zinvB_{qc}_{pj}")
                    zinvhA = zpool.tile([1, NQ], F16, tag="zinvhA",
                                        name=f"zinvhA_{qc}_{pj}")
                    zinvhB = zpool.tile([1, NQ], F16, tag="zinvhB",
                                        name=f"zinvhB_{qc}_{pj}")
                    znA = zpool.tile([1, NQ], F32, tag="znA",
                                     name=f"znA_{qc}_{pj}")
                    znB = zpool.tile([1, NQ], F32, tag="znB",
                                     name=f"znB_{qc}_{pj}")
                    nc.vector.tensor_copy(znA[:], yaugA[64:65, :])
                    nc.vector.tensor_copy(znB[:], yaugB[64:65, :])
                    nc.vector.reciprocal_approx_fast(zinvA[:], znA[:])
                    nc.vector.reciprocal_approx_fast(zinvB[:], znB[:])
                    nc.vector.tensor_copy(zinvhA[:], zinvA[:])
                    nc.vector.tensor_copy(zinvhB[:], zinvB[:])
                    zbA = zpool.tile([64, NQ], F16, tag="zbA",
                                     name=f"zbA_{qc}_{pj}")
                    zbB = zpool.tile([64, NQ], F16, tag="zbB",
                                     name=f"zbB_{qc}_{pj}")
                    nc.gpsimd.partition_broadcast(zbA[:], zinvhA[:],
                                                  channels=64)
                    nc.gpsimd.partition_broadcast(zbB[:], zinvhB[:],
                                                  channels=64)
                    nc.vector.tensor_mul(yq[pj][0:64, :], yaugA[0:64, :],
                                         zbA[:])
                    nc.vector.tensor_mul(yq[pj][64:128, :], yaugB[0:64, :],
                                         zbB[:])

                def new_yq(qc):
                    yq_all[qc] = [ypool.tile([128, NQ], F16, tag=f"yq{g}",
                                             name=f"yq{g}_{qc}")
                                  for g in range(4)]

                # ---- emission schedule ----
                x0 = load_x(0)
                x1 = load_x(1)
                proj_chunk(0, x0)
                x2 = load_x(2)
                proj_chunk(1, x1)
                new_yq(0)
                attn_pj(0, 0)
                x3 = load_x(3)
                proj_chunk(2, x2)
                attn_pj(0, 1)
                proj_chunk(3, x3)
                attn_pj(0, 2)
                attn_pj(0, 3)
                for qc in range(1, NCH):
                    new_yq(qc)
                    state = {"it": 0, "m": 0}
                    sprinkle_every = qc + 1

                    def on_iter(qc=qc, state=state, se=sprinkle_every):
                        state["it"] += 1
                        if state["m"] < 16 and state["it"] % se == 0:
                            oproj_block(qc - 1, state["m"])
                            state["m"] += 1

                    for pj in range(4):
                        attn_pj(qc, pj, on_iter)
                    while state["m"] < 16:
                        oproj_block(qc - 1, state["m"])
                        state["m"] += 1
                for m in range(16):
                    oproj_block(NCH - 1, m)
    nc.finalize()
    return nc


def _rope_tables():
    inv = 1.0 / (ROPE_BASE ** (np.arange(0, D, 2, dtype=np.float32) / D))
    fr_ = np.arange(T, dtype=np.float32)[:, None] * inv[None, :]
    cosT = np.cos(fr_).T.astype(np.float32)
    sinT = np.sin(fr_).T.astype(np.float32)
    cosfull = np.ascontiguousarray(np.tile(cosT, (4, 1)))
    sinfull = np.ascontiguousarray(np.concatenate([-sinT, sinT, -sinT, sinT]))
    return cosfull, sinfull


def _perm_matrix():
    p = np.zeros((128, 128), dtype=np.float32)
    for i in range(128):
        j = i + 32 if (i % 64) < 32 else i - 32
        p[i, j] = 1.0
    return p


def _get_nc():
    if "nc" not in _CACHE:
        _CACHE["nc"] = _build_nc()
    return _CACHE["nc"]


def make_in_maps(x, Wq, Wk, Wv, Wo):
    cosfull, sinfull = _rope_tables()
    permm = _perm_matrix()
    in_maps = []
    def sb_layout(w):
        # [KT*128, M] -> [128, KT*M]: host-side version of the SBUF layout
        kt = w.shape[0] // 128
        return np.ascontiguousarray(
            w.reshape(kt, 128, -1).transpose(1, 0, 2).reshape(128, -1))

    xts = {}
    for b in range(B):
        # [C, T] -> [128, NCH, KT, NQ] so each chunk DMA is contiguous
        xt = x[b].T.reshape(KT, 128, NCH, NQ).transpose(1, 2, 0, 3)
        xts[b] = np.ascontiguousarray(xt).astype(np.float16)
    for c in range(8):
        b, r = divmod(c, 4)
        qcols = np.concatenate(
            [np.arange(64 * (8 * r + h), 64 * (8 * r + h) + 64) for h in LPERM])
        in_maps.append({
            "xT": xts[b],
            "wq": sb_layout(Wq[:, qcols]).astype(np.float16),
            "wk": sb_layout(Wk[:, 128 * r:128 * (r + 1)]).astype(np.float16),
            "wv": sb_layout(Wv[:, 128 * r:128 * (r + 1)]).astype(np.float16),
            "wo": sb_layout(Wo[qcols, :]).astype(np.float16),
            "cosf": cosfull.astype(np.float16),
            "sinf": sinfull.astype(np.float16),
            "perm": permm.astype(np.float16),
        })
    return in_maps


def run(x, Wq, Wk, Wv, Wo, **spmd_kwargs):
    from concourse.bass_utils import run_bass_kernel_spmd

    nc = _get_nc()
    in_maps = make_in_maps(x, Wq, Wk, Wv, Wo)
    res = run_bass_kernel_spmd(nc, in_maps, list(range(8)), **spmd_kwargs)
    out = np.zeros((B, T, C), dtype=np.float32)
    for c in range(8):
        out[c // 4] += res.results[c]["outT"].T.astype(np.float32)
    return out, res


def kernel(**inputs):
    x = np.asarray(inputs["x"], dtype=np.float32)
    Wq = np.asarray(inputs["Wq"], dtype=np.float32)
    Wk = np.asarray(inputs["Wk"], dtype=np.float32)
    Wv = np.asarray(inputs["Wv"], dtype=np.float32)
    Wo = np.asarray(inputs["Wo"], dtype=np.float32)
    out, _ = run(x, Wq, Wk, Wv, Wo)
    return out


# revision 25
# speedup vs baseline: 1.1988x; 1.0137x over previous
"""GQA attention kernel for 8 Trainium2 NeuronCores.

Sharding: 2-way data parallel over batch x 4-way tensor parallel over heads.
Each core handles one batch element and 8 q-heads (2 kv-heads). The o-proj
partial outputs are summed on the host (replaces the all-reduce).

Per-core layout strategy: everything is kept transposed ([feature, seq]) so
every matmul consumes operands directly with the contraction dim on SBUF
partitions and no on-device transposes of activations are needed:
  Q^T = Wq_s^T @ x^T         (lhsT = Wq_s tiles, rhs = x^T tiles)
  S^T[k,q] = K^T_tile^T @ Q^T (k on partitions -> softmax denom via matmul)
  Y^T[d,q] = V_aug^T @ exp(S^T)  (V augmented with a ones column gives the
                                  softmax denominator for free in row 64)
  O^T = Wo_s^T @ (Y^T / Z)

Pipeline structure (v4):
 - single in-order PE stream: proj chunk 0,1 / attn(qc0) interleaved with
   proj chunks 2,3 / attn(qc1..3) with o-proj(qc-1) m-blocks sprinkled
   between attention iterations, so the PE never drains while the scalar
   engine works through the exp stream (exp is the attention-phase pacer).
 - AV matmuls run one kt behind the scores matmuls: the in-order PE queue
   never waits on the exp of the tile it just produced.
 - causality at 128-key granularity: diagonal blocks only compute the
   q >= key part (column-sliced matmul/exp/mask), upper-left full blocks.
 - softmax normalization fully off the PE queue: approx reciprocal on DVE,
   partition broadcast on gpsimd, fused multiply from PSUM on DVE.
 - PSUM: shared 2-buf accumulator pool (proj chains, rope, V-transpose,
   o-proj) + 2x2-bank score tiles + 2 AV accumulators = 8 banks.
"""

import numpy as np

B, T, C, D = 2, 2048, 2048, 64
KT = 16          # contraction tiles over C
NCH = 4          # 512-wide chunks over T
NQ = 512
ROPE_BASE = 10000.0
LPERM = [0, 4, 1, 5, 2, 6, 3, 7]  # local head order: pair j = (j, j+4)

_CACHE = {}


def _build_nc():
    import concourse.bass as bass  # noqa: F401
    import concourse.mybir as mybir
    from concourse import bacc
    from concourse.tile import TileContext
    from concourse.masks import make_identity

    F32 = mybir.dt.float32
    F16 = mybir.dt.float16
    AF = mybir.ActivationFunctionType

    nc = bacc.Bacc(None, target_bir_lowering=False, debug=True)
    # all weight/activation inputs are pre-arranged on the host into the
    # exact SBUF layout so every DMA is a fully contiguous big-run copy
    xT = nc.dram_tensor("xT", [128, NCH, KT, NQ], F16, kind="ExternalInput")
    wq = nc.dram_tensor("wq", [128, KT * 512], F16, kind="ExternalInput")
    wk = nc.dram_tensor("wk", [128, KT * 128], F16, kind="ExternalInput")
    wv = nc.dram_tensor("wv", [128, KT * 128], F16, kind="ExternalInput")
    wo = nc.dram_tensor("wo", [128, 4 * 2048], F16, kind="ExternalInput")
    cosf = nc.dram_tensor("cosf", [128, T], F16, kind="ExternalInput")
    sinf = nc.dram_tensor("sinf", [128, T], F16, kind="ExternalInput")
    perm = nc.dram_tensor("perm", [128, 128], F16, kind="ExternalInput")
    outT = nc.dram_tensor("outT", [C, T], F16, kind="ExternalOutput")

    with TileContext(nc) as tc:
        with (
            tc.tile_pool(name="const", bufs=1) as cpool,
            tc.tile_pool(name="big", bufs=1) as bpool,
        ):
            wq_sb = cpool.tile([128, KT * 512], F16, tag="wq")
            wk_sb = cpool.tile([128, KT * 128], F16, tag="wk")
            wv_sb = cpool.tile([128, KT * 128], F16, tag="wv")
            cos_sb = cpool.tile([128, T], F16, tag="cos")
            sin_sb = cpool.tile([128, T], F16, tag="sin")
            perm_sb = cpool.tile([128, 128], F16, tag="perm")
            ident = cpool.tile([128, 128], F16, tag="ident")
            wo_sb = cpool.tile([128, 4 * 2048], F16, tag="wo")

            nc.sync.dma_start(out=wk_sb[:], in_=wk[:, :])
            nc.sync.dma_start(out=wv_sb[:], in_=wv[:, :])
            make_identity(nc, ident[:])
            ebias = cpool.tile([128, 1], F32, tag="ebias")
            nc.vector.memset(ebias[:], -8.0)

            # persistent transposed activations
            qt = [bpool.tile([128, T], F16, tag=f"qt{j}", name=f"qt{j}") for j in range(4)]
            kt_sb = bpool.tile([128, T], F16, tag="ktT")
            vnat = bpool.tile([128, KT * 130], F16, tag="vnat")
            nc.vector.memset(vnat[:], 1.0)

            with (
                tc.tile_pool(name="xs", bufs=4) as xpool,
                tc.tile_pool(name="acc", bufs=2, space="PSUM") as acc,
                tc.tile_pool(name="sps", bufs=2, space="PSUM") as spool,
                tc.tile_pool(name="aps", bufs=1, space="PSUM") as apool,
                tc.tile_pool(name="rtmp", bufs=4) as rtpool,
                tc.tile_pool(name="vtt", bufs=2) as vtpool,
                tc.tile_pool(name="esb", bufs=5) as epool,
                tc.tile_pool(name="ysb", bufs=2) as ypool,
                tc.tile_pool(name="zsb", bufs=2) as zpool,
                tc.tile_pool(name="stg", bufs=3) as stpool,
            ):
                yq_all = [None] * NCH

                def load_x(n):
                    xsb = []
                    for half in range(2):
                        xh = xpool.tile([128, 8 * NQ], F16, tag="xsb")
                        nc.sync.dma_start(
                            out=xh[:].rearrange("p (kt t) -> p kt t", kt=8),
                            in_=xT[:, n, half * 8:(half + 1) * 8, :],
                        )
                        xsb.append(xh)
                    return xsb

                def proj_chunk(n, xsb):
                    nsl = slice(n * NQ, (n + 1) * NQ)
                    if n == 0:
                        # everything below is needed only from the rope
                        # phase / attention onward; issued after the
                        # hot-path wk/wv/x DMAs so startup isn't delayed
                        nc.sync.dma_start(out=wq_sb[:], in_=wq[:, :])
                        nc.sync.dma_start(out=perm_sb[:], in_=perm[:, :])
                        nc.sync.dma_start(out=cos_sb[:], in_=cosf[:, :])
                        nc.sync.dma_start(out=sin_sb[:], in_=sinf[:, :])
                        nc.sync.dma_start(out=wo_sb[:], in_=wo[:, :])
                    vt_sb = vtpool.tile([128, NQ], F16, tag="vtT")
                    # K and V chains first: their weights arrive first and V
                    # feeds the vnat transposes
                    for m in (4, 5, 0, 1, 2, 3):
                        ps = acc.tile([128, NQ], F32, tag="ps")
                        for kt in range(KT):
                            if m < 4:
                                w_ap = wq_sb[:, kt * 512 + m * 128: kt * 512 + (m + 1) * 128]
                            elif m == 4:
                                w_ap = wk_sb[:, kt * 128:(kt + 1) * 128]
                            else:
                                w_ap = wv_sb[:, kt * 128:(kt + 1) * 128]
                            x_ap = xsb[kt // 8][:, (kt % 8) * NQ:(kt % 8 + 1) * NQ]
                            nc.tensor.matmul(
                                ps[:], w_ap, x_ap,
                                start=(kt == 0), stop=(kt == KT - 1),
                                skip_group_check=True,
                            )
                        if m < 4:
                            nc.scalar.copy(qt[m][:, nsl], ps[:])
                        elif m == 4:
                            nc.scalar.copy(kt_sb[:, nsl], ps[:])
                        else:
                            nc.scalar.copy(vt_sb[:], ps[:])
                        if m == 5:
                            # V chunk -> vnat transposed blocks
                            for j in range(4):
                                g = 4 * n + j
                                tp = acc.tile([128, NQ], F32, tag="ps")
                                tpv = tp[:, 0:64].bitcast(F16)
                                nc.tensor.transpose(
                                    tpv,
                                    vt_sb[:, j * 128:(j + 1) * 128], ident[:])
                                nc.vector.tensor_copy(
                                    vnat[:, g * 130: g * 130 + 64], tpv[:, 0:64])
                                nc.vector.tensor_copy(
                                    vnat[:, g * 130 + 65: g * 130 + 129],
                                    tpv[:, 64:128])
                    # rope on this chunk of q0..q3 and k
                    for tile in [qt[0], qt[1], qt[2], qt[3], kt_sb]:
                        qs = acc.tile([128, NQ], F32, tag="ps")
                        nc.tensor.matmul(qs[:], perm_sb[:], tile[:, nsl],
                                         start=True, stop=True,
                                         skip_group_check=True)
                        t1 = rtpool.tile([128, NQ], F16, tag="t1")
                        t2 = rtpool.tile([128, NQ], F16, tag="t2")
                        nc.vector.tensor_mul(t1[:], tile[:, nsl], cos_sb[:, nsl])
                        nc.vector.tensor_mul(t2[:], qs[:], sin_sb[:, nsl])
                        nc.vector.tensor_add(tile[:, nsl], t1[:], t2[:])

                def oproj_block(qc, m):
                    qsl = slice(qc * NQ, (qc + 1) * NQ)
                    ops = acc.tile([128, NQ], F32, tag="ps")
                    for g in range(4):
                        nc.tensor.matmul(
                            ops[:],
                            wo_sb[:, g * 2048 + m * 128: g * 2048 + (m + 1) * 128],
                            yq_all[qc][g][:], start=(g == 0), stop=(g == 3),
                            skip_group_check=True)
                    st = stpool.tile([128, NQ], F16, tag="st")
                    nc.vector.tensor_copy(st[:], ops[:])
                    nc.sync.dma_start(
                        out=outT[m * 128:(m + 1) * 128, qsl], in_=st[:])

                def attn_pj(qc, pj, on_iter=None):
                    qbase = qc * NQ
                    kt_hi = 4 * (qc + 1)
                    yaugA = apool.tile([65, NQ], F32, tag="yaugA",
                                       name=f"yaugA_{qc}_{pj}")
                    yaugB = apool.tile([65, NQ], F32, tag="yaugB",
                                       name=f"yaugB_{qc}_{pj}")

                    def av_mms(eab, kt, off):
                        nc.tensor.matmul(
                            yaugA[:, off:NQ], vnat[:, kt * 130: kt * 130 + 65],
                            eab[:, off:NQ],
                            start=(kt == 0), stop=(kt == kt_hi - 1),
                            skip_group_check=True)
                        nc.tensor.matmul(
                            yaugB[:, off:NQ],
                            vnat[:, kt * 130 + 65: kt * 130 + 130],
                            eab[:, NQ + off:2 * NQ],
                            start=(kt == 0), stop=(kt == kt_hi - 1),
                            skip_group_check=True)

                    # AV runs two kt behind the scores: the in-order PE queue
                    # never waits on the exp of the tile just produced, and at
                    # a pj boundary the first AV (WAR on the reused yaug bank)
                    # is reached only after the previous pj's normalize is done
                    pending = []
                    for kt in range(kt_hi):
                        diag = kt * 128 >= qbase
                        off = max(0, kt * 128 - qbase)
                        w = NQ - off
                        sab = spool.tile([128, 2 * NQ], F32, tag="sab",
                                         name=f"sab_{qc}_{pj}_{kt}")
                        nc.tensor.matmul(
                            sab[:, off:NQ],
                            kt_sb[0:64, kt * 128:(kt + 1) * 128],
                            qt[pj][0:64, qbase + off:qbase + NQ],
                            start=True, stop=True, skip_group_check=True)
                        nc.tensor.matmul(
                            sab[:, NQ + off:2 * NQ],
                            kt_sb[64:128, kt * 128:(kt + 1) * 128],
                            qt[pj][64:128, qbase + off:qbase + NQ],
                            start=True, stop=True, skip_group_check=True)
                        if len(pending) >= 2:
                            av_mms(*pending.pop(0))
                        eab = epool.tile([128, 2 * NQ], F16, tag="eab",
                                         name=f"eab_{qc}_{pj}_{kt}")
                        eview = eab[:].rearrange("p (h q) -> p h q", h=2)[:, :, off:NQ]
                        sview = sab[:].rearrange("p (h q) -> p h q", h=2)[:, :, off:NQ]
                        nc.scalar.activation(eview, sview, AF.Exp,
                                             scale=0.125, bias=ebias[:])
                        if diag:
                            nc.gpsimd.affine_select(
                                out=eview, in_=eview,
                                compare_op=mybir.AluOpType.is_ge,
                                fill=0.0,
                                base=0,
                                channel_multiplier=-1,
                                pattern=[[0, 2], [1, w]],
                            )
                        pending.append((eab, kt, off))
                        if on_iter is not None:
                            on_iter()
                    for p in pending:
                        av_mms(*p)
                    # normalize pj: 1/Z on DVE, broadcast on gpsimd
                    yq = yq_all[qc]
                    zinvA = zpool.tile([1, NQ], F32, tag="zinvA",
                                       name=f"zinvA_{qc}_{pj}")
                    zinvB = zpool.tile([1, NQ], F32, tag="zinvB",
                                       name=f"zinvB_{qc}_{pj}")
                    zinvhA = zpool.tile([1, NQ], F16, tag="zinvhA",
                                        name=f"zinvhA_{qc}_{pj}")
                    zinvhB = zpool.tile([1, NQ], F16, tag="zinvhB",
                                        name=f"zinvhB_{qc}_{pj}")
                    znA = zpool.tile([1, NQ], F32, tag="znA",
                                     name=f"znA_{qc}_{pj}")
                    znB = zpool.tile([1, NQ], F32, tag="znB",
                                     name=f"znB_{qc}_{pj}")
                    nc.vector.tensor_copy(znA[:], yaugA[64:65, :])
                    nc.vector.tensor_copy(znB[:], yaugB[64:65, :])
                    nc.vector.reciprocal_approx_fast(zinvA[:], znA[:])
                    nc.vector.reciprocal_approx_fast(zinvB[:], znB[:])
                    nc.vector.tensor_copy(zinvhA[:], zinvA[:])
                    nc.vector.tensor_copy(zinvhB[:], zinvB[:])
                    zbA = zpool.tile([64, NQ], F16, tag="zbA",
                                     name=f"zbA_{qc}_{pj}")
                    zbB = zpool.tile([64, NQ], F16, tag="zbB",
                                     name=f"zbB_{qc}_{pj}")
                    nc.gpsimd.partition_broadcast(zbA[:], zinvhA[:],
                                                  channels=64)
                    nc.gpsimd.partition_broadcast(zbB[:], zinvhB[:],
                                                  channels=64)
                    nc.vector.tensor_mul(yq[pj][0:64, :], yaugA[0:64, :],
                                         zbA[:])
                    nc.vector.tensor_mul(yq[pj][64:128, :], yaugB[0:64, :],
                                         zbB[:])

                def new_yq(qc):
                    yq_all[qc] = [ypool.tile([128, NQ], F16, tag=f"yq{g}",
                                             name=f"yq{g}_{qc}")
                                  for g in range(4)]

                # ---- emission schedule ----
                x0 = load_x(0)
                x1 = load_x(1)
                proj_chunk(0, x0)
                x2 = load_x(2)
                proj_chunk(1, x1)
                new_yq(0)
                attn_pj(0, 0)
                x3 = load_x(3)
                proj_chunk(2, x2)
                attn_pj(0, 1)
                proj_chunk(3, x3)
                attn_pj(0, 2)
                attn_pj(0, 3)
                for qc in range(1, NCH):
                    new_yq(qc)
                    state = {"it": 0, "m": 0}
                    sprinkle_every = qc + 1

                    def on_iter(qc=qc, state=state, se=sprinkle_every):
                        state["it"] += 1
                        if state["m"] < 16 and state["it"] % se == 0:
                            oproj_block(qc - 1, state["m"])
                            state["m"] += 1

                    for pj in range(4):
                        attn_pj(qc, pj, on_iter)
                    while state["m"] < 16:
                        oproj_block(qc - 1, state["m"])
                        state["m"] += 1
                for m in range(16):
                    oproj_block(NCH - 1, m)
    nc.finalize()
    return nc


def _rope_tables():
    inv = 1.0 / (ROPE_BASE ** (np.arange(0, D, 2, dtype=np.float32) / D))
    fr_ = np.arange(T, dtype=np.float32)[:, None] * inv[None, :]
    cosT = np.cos(fr_).T.astype(np.float32)
    sinT = np.sin(fr_).T.astype(np.float32)
    cosfull = np.ascontiguousarray(np.tile(cosT, (4, 1)))
    sinfull = np.ascontiguousarray(np.concatenate([-sinT, sinT, -sinT, sinT]))
    return cosfull, sinfull


def _perm_matrix():
    p = np.zeros((128, 128), dtype=np.float32)
    for i in range(128):
        j = i + 32 if (i % 64) < 32 else i - 32
        p[i, j] = 1.0
    return p


def _get_nc():
    if "nc" not in _CACHE:
        _CACHE["nc"] = _build_nc()
    return _CACHE["nc"]


def make_in_maps(x, Wq, Wk, Wv, Wo):
    cosfull, sinfull = _rope_tables()
    permm = _perm_matrix()
    in_maps = []
    def sb_layout(w):
        # [KT*128, M] -> [128, KT*M]: host-side version of the SBUF layout
        kt = w.shape[0] // 128
        return np.ascontiguousarray(
            w.reshape(kt, 128, -1).transpose(1, 0, 2).reshape(128, -1))

    xts = {}
    for b in range(B):
        # [C, T] -> [128, NCH, KT, NQ] so each chunk DMA is contiguous
        xt = x[b].T.reshape(KT, 128, NCH, NQ).transpose(1, 2, 0, 3)
        xts[b] = np.ascontiguousarray(xt).astype(np.float16)
    for c in range(8):
        b, r = divmod(c, 4)
        qcols = np.concatenate(
            [np.arange(64 * (8 * r + h), 64 * (8 * r + h) + 64) for h in LPERM])
        in_maps.append({
            "xT": xts[b],
            "wq": sb_layout(Wq[:, qcols]).astype(np.float16),
            "wk": sb_layout(Wk[:, 128 * r:128 * (r + 1)]).astype(np.float16),
            "wv": sb_layout(Wv[:, 128 * r:128 * (r + 1)]).astype(np.float16),
            "wo": sb_layout(Wo[qcols, :]).astype(np.float16),
            "cosf": cosfull.astype(np.float16),
            "sinf": sinfull.astype(np.float16),
            "perm": permm.astype(np.float16),
        })
    return in_maps


def run(x, Wq, Wk, Wv, Wo, **spmd_kwargs):
    from concourse.bass_utils import run_bass_kernel_spmd

    nc = _get_nc()
    in_maps = make_in_maps(x, Wq, Wk, Wv, Wo)
    res = run_bass_kernel_spmd(nc, in_maps, list(range(8)), **spmd_kwargs)
    out = np.zeros((B, T, C), dtype=np.float32)
    for c in range(8):
        out[c // 4] += res.results[c]["outT"].T.astype(np.float32)
    return out, res


def kernel(**inputs):
    x = np.asarray(inputs["x"], dtype=np.float32)
    Wq = np.asarray(inputs["Wq"], dtype=np.float32)
    Wk = np.asarray(inputs["Wk"], dtype=np.float32)
    Wv = np.asarray(inputs["Wv"], dtype=np.float32)
    Wo = np.asarray(inputs["Wo"], dtype=np.float32)
    out, _ = run(x, Wq, Wk, Wv, Wo)
    return out


# revision 28
# speedup vs baseline: 1.2414x; 1.0355x over previous
"""GQA attention kernel for 8 Trainium2 NeuronCores.

Sharding: 2-way data parallel over batch x 4-way tensor parallel over heads.
Each core handles one batch element and 8 q-heads (2 kv-heads). The o-proj
partial outputs are summed on the host (replaces the all-reduce).

Per-core layout strategy: everything is kept transposed ([feature, seq]) so
every matmul consumes operands directly with the contraction dim on SBUF
partitions and no on-device transposes of activations are needed:
  Q^T = Wq_s^T @ x^T         (lhsT = Wq_s tiles, rhs = x^T tiles)
  S^T[k,q] = K^T_tile^T @ Q^T (k on partitions -> softmax denom via matmul)
  Y^T[d,q] = V_aug^T @ exp(S^T)  (V augmented with a ones column gives the
                                  softmax denominator for free in row 64)
  O^T = Wo_s^T @ (Y^T / Z)

Pipeline structure (v4):
 - single in-order PE stream: proj chunk 0,1 / attn(qc0) interleaved with
   proj chunks 2,3 / attn(qc1..3) with o-proj(qc-1) m-blocks sprinkled
   between attention iterations, so the PE never drains while the scalar
   engine works through the exp stream (exp is the attention-phase pacer).
 - AV matmuls run one kt behind the scores matmuls: the in-order PE queue
   never waits on the exp of the tile it just produced.
 - causality at 128-key granularity: diagonal blocks only compute the
   q >= key part (column-sliced matmul/exp/mask), upper-left full blocks.
 - softmax normalization fully off the PE queue: approx reciprocal on DVE,
   partition broadcast on gpsimd, fused multiply from PSUM on DVE.
 - PSUM: shared 2-buf accumulator pool (proj chains, rope, V-transpose,
   o-proj) + 2x2-bank score tiles + 2 AV accumulators = 8 banks.
"""

import numpy as np

B, T, C, D = 2, 2048, 2048, 64
KT = 16          # contraction tiles over C
NCH = 4          # 512-wide chunks over T
NQ = 512
ROPE_BASE = 10000.0
LPERM = [0, 4, 1, 5, 2, 6, 3, 7]  # local head order: pair j = (j, j+4)

_CACHE = {}


def _build_nc():
    import concourse.bass as bass  # noqa: F401
    import concourse.mybir as mybir
    from concourse import bacc
    from concourse.tile import TileContext
    from concourse.masks import make_identity

    F32 = mybir.dt.float32
    F16 = mybir.dt.float16
    AF = mybir.ActivationFunctionType

    nc = bacc.Bacc(None, target_bir_lowering=False, debug=True)
    # all weight/activation inputs are pre-arranged on the host into the
    # exact SBUF layout so every DMA is a fully contiguous big-run copy
    xT = nc.dram_tensor("xT", [128, NCH, KT, NQ], F16, kind="ExternalInput")
    wq = nc.dram_tensor("wq", [128, KT * 512], F16, kind="ExternalInput")
    wk = nc.dram_tensor("wk", [128, KT * 128], F16, kind="ExternalInput")
    wv = nc.dram_tensor("wv", [128, KT * 128], F16, kind="ExternalInput")
    wo = nc.dram_tensor("wo", [128, 4 * 2048], F16, kind="ExternalInput")
    cosf = nc.dram_tensor("cosf", [128, T], F16, kind="ExternalInput")
    sinf = nc.dram_tensor("sinf", [128, T], F16, kind="ExternalInput")
    perm = nc.dram_tensor("perm", [128, 128], F16, kind="ExternalInput")
    outT = nc.dram_tensor("outT", [C, T], F16, kind="ExternalOutput")

    with TileContext(nc) as tc:
        with (
            tc.tile_pool(name="const", bufs=1) as cpool,
            tc.tile_pool(name="big", bufs=1) as bpool,
        ):
            wq_sb = cpool.tile([128, KT * 512], F16, tag="wq")
            wk_sb = cpool.tile([128, KT * 128], F16, tag="wk")
            wv_sb = cpool.tile([128, KT * 128], F16, tag="wv")
            cos_sb = cpool.tile([128, T], F16, tag="cos")
            sin_sb = cpool.tile([128, T], F16, tag="sin")
            perm_sb = cpool.tile([128, 128], F16, tag="perm")
            ident = cpool.tile([128, 128], F16, tag="ident")
            wo_sb = cpool.tile([128, 4 * 2048], F16, tag="wo")

            nc.sync.dma_start(out=wk_sb[:], in_=wk[:, :])
            nc.sync.dma_start(out=wv_sb[:], in_=wv[:, :])
            make_identity(nc, ident[:])
            ebias = cpool.tile([128, 1], F32, tag="ebias")
            nc.vector.memset(ebias[:], -8.0)

            # persistent transposed activations
            qt = [bpool.tile([128, T], F16, tag=f"qt{j}", name=f"qt{j}") for j in range(4)]
            kt_sb = bpool.tile([128, T], F16, tag="ktT")
            vnat = bpool.tile([128, KT * 130], F16, tag="vnat")
            nc.vector.memset(vnat[:], 1.0)

            with (
                tc.tile_pool(name="xs", bufs=4) as xpool,
                tc.tile_pool(name="acc", bufs=2, space="PSUM") as acc,
                tc.tile_pool(name="sps", bufs=2, space="PSUM") as spool,
                tc.tile_pool(name="aps", bufs=1, space="PSUM") as apool,
                tc.tile_pool(name="rtmp", bufs=4) as rtpool,
                tc.tile_pool(name="vtt", bufs=2) as vtpool,
                tc.tile_pool(name="esb", bufs=5) as epool,
                tc.tile_pool(name="ysb", bufs=2) as ypool,
                tc.tile_pool(name="zsb", bufs=2) as zpool,
                tc.tile_pool(name="stg", bufs=3) as stpool,
            ):
                yq_all = [None] * NCH

                def load_x(n):
                    xsb = []
                    for half in range(2):
                        xh = xpool.tile([128, 8 * NQ], F16, tag="xsb")
                        nc.sync.dma_start(
                            out=xh[:].rearrange("p (kt t) -> p kt t", kt=8),
                            in_=xT[:, n, half * 8:(half + 1) * 8, :],
                        )
                        xsb.append(xh)
                    return xsb

                def proj_chunk(n, xsb):
                    nsl = slice(n * NQ, (n + 1) * NQ)
                    if n == 0:
                        # everything below is needed only from the rope
                        # phase / attention onward; issued after the
                        # hot-path wk/wv/x DMAs so startup isn't delayed
                        nc.sync.dma_start(out=perm_sb[:], in_=perm[:, :])
                        nc.sync.dma_start(out=cos_sb[:], in_=cosf[:, :])
                        nc.sync.dma_start(out=sin_sb[:], in_=sinf[:, :])
                        nc.sync.dma_start(out=wo_sb[:], in_=wo[:, :])
                    vt_sb = vtpool.tile([128, NQ], F16, tag="vtT")
                    # K and V chains first: their weights arrive first and V
                    # feeds the vnat transposes
                    for m in (4, 5, 0, 1, 2, 3):
                        ps = acc.tile([128, NQ], F32, tag="ps")
                        for kt in range(KT):
                            if m < 4:
                                w_ap = wq_sb[:, kt * 512 + m * 128: kt * 512 + (m + 1) * 128]
                            elif m == 4:
                                w_ap = wk_sb[:, kt * 128:(kt + 1) * 128]
                            else:
                                w_ap = wv_sb[:, kt * 128:(kt + 1) * 128]
                            x_ap = xsb[kt // 8][:, (kt % 8) * NQ:(kt % 8 + 1) * NQ]
                            nc.tensor.matmul(
                                ps[:], w_ap, x_ap,
                                start=(kt == 0), stop=(kt == KT - 1),
                                skip_group_check=True,
                            )
                        if m < 4:
                            nc.scalar.copy(qt[m][:, nsl], ps[:])
                        elif m == 4:
                            nc.scalar.copy(kt_sb[:, nsl], ps[:])
                        else:
                            nc.scalar.copy(vt_sb[:], ps[:])
                        if m == 5:
                            # V chunk -> vnat transposed blocks
                            for j in range(4):
                                g = 4 * n + j
                                tp = acc.tile([128, NQ], F32, tag="ps")
                                tpv = tp[:, 0:64].bitcast(F16)
                                nc.tensor.transpose(
                                    tpv,
                                    vt_sb[:, j * 128:(j + 1) * 128], ident[:])
                                nc.vector.tensor_copy(
                                    vnat[:, g * 130: g * 130 + 64], tpv[:, 0:64])
                                nc.vector.tensor_copy(
                                    vnat[:, g * 130 + 65: g * 130 + 129],
                                    tpv[:, 64:128])
                    # rope on this chunk of q0..q3 and k
                    for tile in [qt[0], qt[1], qt[2], qt[3], kt_sb]:
                        qs = acc.tile([128, NQ], F32, tag="ps")
                        nc.tensor.matmul(qs[:], perm_sb[:], tile[:, nsl],
                                         start=True, stop=True,
                                         skip_group_check=True)
                        t1 = rtpool.tile([128, NQ], F16, tag="t1")
                        t2 = rtpool.tile([128, NQ], F16, tag="t2")
                        nc.vector.tensor_mul(t1[:], tile[:, nsl], cos_sb[:, nsl])
                        nc.vector.tensor_mul(t2[:], qs[:], sin_sb[:, nsl])
                        nc.vector.tensor_add(tile[:, nsl], t1[:], t2[:])

                def oproj_block(qc, m):
                    qsl = slice(qc * NQ, (qc + 1) * NQ)
                    ops = acc.tile([128, NQ], F32, tag="ps")
                    for g in range(4):
                        nc.tensor.matmul(
                            ops[:],
                            wo_sb[:, g * 2048 + m * 128: g * 2048 + (m + 1) * 128],
                            yq_all[qc][g][:], start=(g == 0), stop=(g == 3),
                            skip_group_check=True)
                    st = stpool.tile([128, NQ], F16, tag="st")
                    nc.vector.tensor_copy(st[:], ops[:])
                    nc.sync.dma_start(
                        out=outT[m * 128:(m + 1) * 128, qsl], in_=st[:])

                def attn_pj(qc, pj, on_iter=None):
                    qbase = qc * NQ
                    kt_hi = 4 * (qc + 1)
                    yaugA = apool.tile([65, NQ], F32, tag="yaugA",
                                       name=f"yaugA_{qc}_{pj}")
                    yaugB = apool.tile([65, NQ], F32, tag="yaugB",
                                       name=f"yaugB_{qc}_{pj}")

                    def av_mms(eab, kt, off):
                        nc.tensor.matmul(
                            yaugA[:, off:NQ], vnat[:, kt * 130: kt * 130 + 65],
                            eab[:, off:NQ],
                            start=(kt == 0), stop=(kt == kt_hi - 1),
                            skip_group_check=True)
                        nc.tensor.matmul(
                            yaugB[:, off:NQ],
                            vnat[:, kt * 130 + 65: kt * 130 + 130],
                            eab[:, NQ + off:2 * NQ],
                            start=(kt == 0), stop=(kt == kt_hi - 1),
                            skip_group_check=True)

                    # AV runs two kt behind the scores: the in-order PE queue
                    # never waits on the exp of the tile just produced, and at
                    # a pj boundary the first AV (WAR on the reused yaug bank)
                    # is reached only after the previous pj's normalize is done
                    pending = []
                    for kt in range(kt_hi):
                        diag = kt * 128 >= qbase
                        off = max(0, kt * 128 - qbase)
                        w = NQ - off
                        sab = spool.tile([128, 2 * NQ], F32, tag="sab",
                                         name=f"sab_{qc}_{pj}_{kt}")
                        nc.tensor.matmul(
                            sab[:, off:NQ],
                            kt_sb[0:64, kt * 128:(kt + 1) * 128],
                            qt[pj][0:64, qbase + off:qbase + NQ],
                            start=True, stop=True, skip_group_check=True)
                        nc.tensor.matmul(
                            sab[:, NQ + off:2 * NQ],
                            kt_sb[64:128, kt * 128:(kt + 1) * 128],
                            qt[pj][64:128, qbase + off:qbase + NQ],
                            start=True, stop=True, skip_group_check=True)
                        lag = 1 if qc == 0 else 2
                        if len(pending) >= lag:
                            av_mms(*pending.pop(0))
                        eab = epool.tile([128, 2 * NQ], F16, tag="eab",
                                         name=f"eab_{qc}_{pj}_{kt}")
                        eview = eab[:].rearrange("p (h q) -> p h q", h=2)[:, :, off:NQ]
                        sview = sab[:].rearrange("p (h q) -> p h q", h=2)[:, :, off:NQ]
                        nc.scalar.activation(eview, sview, AF.Exp,
                                             scale=0.125, bias=ebias[:])
                        if diag:
                            nc.gpsimd.affine_select(
                                out=eview, in_=eview,
                                compare_op=mybir.AluOpType.is_ge,
                                fill=0.0,
                                base=0,
                                channel_multiplier=-1,
                                pattern=[[0, 2], [1, w]],
                            )
                        pending.append((eab, kt, off))
                        if on_iter is not None:
                            on_iter()
                    for p in pending:
                        av_mms(*p)
                    # normalize pj: 1/Z on DVE, broadcast on gpsimd
                    yq = yq_all[qc]
                    zinvA = zpool.tile([1, NQ], F32, tag="zinvA",
                                       name=f"zinvA_{qc}_{pj}")
                    zinvB = zpool.tile([1, NQ], F32, tag="zinvB",
                                       name=f"zinvB_{qc}_{pj}")
                    zinvhA = zpool.tile([1, NQ], F16, tag="zinvhA",
                                        name=f"zinvhA_{qc}_{pj}")
                    zinvhB = zpool.tile([1, NQ], F16, tag="zinvhB",
                                        name=f"zinvhB_{qc}_{pj}")
                    znA = zpool.tile([1, NQ], F32, tag="znA",
                                     name=f"znA_{qc}_{pj}")
                    znB = zpool.tile([1, NQ], F32, tag="znB",
                                     name=f"znB_{qc}_{pj}")
                    nc.vector.tensor_copy(znA[:], yaugA[64:65, :])
                    nc.vector.tensor_copy(znB[:], yaugB[64:65, :])
                    nc.vector.reciprocal_approx_fast(zinvA[:], znA[:])
                    nc.vector.reciprocal_approx_fast(zinvB[:], znB[:])
                    nc.vector.tensor_copy(zinvhA[:], zinvA[:])
                    nc.vector.tensor_copy(zinvhB[:], zinvB[:])
                    zbA = zpool.tile([64, NQ], F16, tag="zbA",
                                     name=f"zbA_{qc}_{pj}")
                    zbB = zpool.tile([64, NQ], F16, tag="zbB",
                                     name=f"zbB_{qc}_{pj}")
                    nc.gpsimd.partition_broadcast(zbA[:], zinvhA[:],
                                                  channels=64)
                    nc.gpsimd.partition_broadcast(zbB[:], zinvhB[:],
                                                  channels=64)
                    nc.vector.tensor_mul(yq[pj][0:64, :], yaugA[0:64, :],
                                         zbA[:])
                    nc.vector.tensor_mul(yq[pj][64:128, :], yaugB[0:64, :],
                                         zbB[:])

                def new_yq(qc):
                    yq_all[qc] = [ypool.tile([128, NQ], F16, tag=f"yq{g}",
                                             name=f"yq{g}_{qc}")
                                  for g in range(4)]

                # ---- emission schedule ----
                x0 = load_x(0)
                nc.sync.dma_start(out=wq_sb[:], in_=wq[:, :])
                x1 = load_x(1)
                proj_chunk(0, x0)
                x2 = load_x(2)
                proj_chunk(1, x1)
                new_yq(0)
                attn_pj(0, 0)
                x3 = load_x(3)
                proj_chunk(2, x2)
                attn_pj(0, 1)
                proj_chunk(3, x3)
                attn_pj(0, 2)
                attn_pj(0, 3)
                for qc in range(1, NCH):
                    new_yq(qc)
                    state = {"it": 0, "m": 0}
                    sprinkle_every = qc + 1

                    def on_iter(qc=qc, state=state, se=sprinkle_every):
                        state["it"] += 1
                        if state["m"] < 16 and state["it"] % se == 0:
                            oproj_block(qc - 1, state["m"])
                            state["m"] += 1

                    for pj in range(4):
                        attn_pj(qc, pj, on_iter)
                    while state["m"] < 16:
                        oproj_block(qc - 1, state["m"])
                        state["m"] += 1
                for m in range(16):
                    oproj_block(NCH - 1, m)
    nc.finalize()
    return nc


def _rope_tables():
    inv = 1.0 / (ROPE_BASE ** (np.arange(0, D, 2, dtype=np.float32) / D))
    fr_ = np.arange(T, dtype=np.float32)[:, None] * inv[None, :]
    cosT = np.cos(fr_).T.astype(np.float32)
    sinT = np.sin(fr_).T.astype(np.float32)
    cosfull = np.ascontiguousarray(np.tile(cosT, (4, 1)))
    sinfull = np.ascontiguousarray(np.concatenate([-sinT, sinT, -sinT, sinT]))
    return cosfull, sinfull


def _perm_matrix():
    p = np.zeros((128, 128), dtype=np.float32)
    for i in range(128):
        j = i + 32 if (i % 64) < 32 else i - 32
        p[i, j] = 1.0
    return p


def _get_nc():
    if "nc" not in _CACHE:
        _CACHE["nc"] = _build_nc()
    return _CACHE["nc"]


def make_in_maps(x, Wq, Wk, Wv, Wo):
    cosfull, sinfull = _rope_tables()
    permm = _perm_matrix()
    in_maps = []
    def sb_layout(w):
        # [KT*128, M] -> [128, KT*M]: host-side version of the SBUF layout
        kt = w.shape[0] // 128
        return np.ascontiguousarray(
            w.reshape(kt, 128, -1).transpose(1, 0, 2).reshape(128, -1))

    xts = {}
    for b in range(B):
        # [C, T] -> [128, NCH, KT, NQ] so each chunk DMA is contiguous
        xt = x[b].T.reshape(KT, 128, NCH, NQ).transpose(1, 2, 0, 3)
        xts[b] = np.ascontiguousarray(xt).astype(np.float16)
    for c in range(8):
        b, r = divmod(c, 4)
        qcols = np.concatenate(
            [np.arange(64 * (8 * r + h), 64 * (8 * r + h) + 64) for h in LPERM])
        in_maps.append({
            "xT": xts[b],
            "wq": sb_layout(Wq[:, qcols]).astype(np.float16),
            "wk": sb_layout(Wk[:, 128 * r:128 * (r + 1)]).astype(np.float16),
            "wv": sb_layout(Wv[:, 128 * r:128 * (r + 1)]).astype(np.float16),
            "wo": sb_layout(Wo[qcols, :]).astype(np.float16),
            "cosf": cosfull.astype(np.float16),
            "sinf": sinfull.astype(np.float16),
            "perm": permm.astype(np.float16),
        })
    return in_maps


def run(x, Wq, Wk, Wv, Wo, **spmd_kwargs):
    from concourse.bass_utils import run_bass_kernel_spmd

    nc = _get_nc()
    in_maps = make_in_maps(x, Wq, Wk, Wv, Wo)
    res = run_bass_kernel_spmd(nc, in_maps, list(range(8)), **spmd_kwargs)
    out = np.zeros((B, T, C), dtype=np.float32)
    for c in range(8):
        out[c // 4] += res.results[c]["outT"].T.astype(np.float32)
    return out, res


def kernel(**inputs):
    x = np.asarray(inputs["x"], dtype=np.float32)
    Wq = np.asarray(inputs["Wq"], dtype=np.float32)
    Wk = np.asarray(inputs["Wk"], dtype=np.float32)
    Wv = np.asarray(inputs["Wv"], dtype=np.float32)
    Wo = np.asarray(inputs["Wo"], dtype=np.float32)
    out, _ = run(x, Wq, Wk, Wv, Wo)
    return out
